# revision 15
# baseline (speedup 1.0000x reference)
"""Trainium2 Bass kernel for the caption-generation module (2-layer GRU
encoder-decoder + vocab projection + log_softmax).

Strategy: data-parallel over batch across 8 NeuronCores (B=128 -> 16 rows
per core, weights replicated).  Per core, everything runs in a transposed
layout (feature dim on SBUF partitions, (time*batch) on the free dim):

  E1:  gi1[t] = x_t @ w_ih1.T for all 40 encoder steps  (one batched matmul)
  C1:  h1 chain, 67 sequential steps, only h1 @ w_hh1.T inside the loop
       (decoder rnn1 input is zero so its gi is just the bias)
  E3:  gi2[t] = [h1_t; w_t] @ w_ih2.T for all 67 steps  (batched matmul)
  C2:  h2 chain, 67 sequential steps
  P :  logits = h2_dec @ out_w.T + out_b, then streamed log_softmax,
       DMA straight to the output

Matmul inputs are cast to bf16 (fp32 accumulate in PSUM); gate math and
softmax run in fp32.
"""

import sys
import types

sys.path.insert(0, "/opt/trn_rl_repo")

import numpy as np
import ml_dtypes

import concourse.bass as bass
import concourse.mybir as mybir
import concourse.tile as tile
from concourse.alu_op_type import AluOpType
from concourse.vector_clock import ScopedClock

BF16 = mybir.dt.bfloat16
F32 = mybir.dt.float32
F8 = mybir.dt.float8e3
WSCALE = 128.0  # fp8 chain-weight pre-scale (host multiplies, gates divide)
AF = mybir.ActivationFunctionType


# ---------------------------------------------------------------------------
# Workaround: this container's walrus rejects CTRL instructions carrying more
# than one sync-wait command.  Split the TileContext tail drain's wait list
# across a chain of drains, one wait each.
# ---------------------------------------------------------------------------
def _patched_drain_and_barrier(self, tick_clock, wait_clock):
    import bass_rust

    drain_inst = self.nc.sync.drain()
    wait_clock.add_sem_waits(
        drain_inst.ins, ScopedClock({None: tick_clock.global_clock})
    )
    waits = list(drain_inst.ins.sync_info.on_wait)
    if len(waits) > 1:
        si = drain_inst.ins.sync_info
        si.on_wait = waits[:1]
        drain_inst.ins.sync_info = si
        for i in range(1, len(waits)):
            extra = self.nc.sync.drain()
            extra.ins.sync_info = bass_rust.SyncInfo(
                on_wait=waits[i : i + 1], on_update=[]
            )
    self.nc.all_engine_barrier()
    assert self.sems is not None
    popped = self.nc._tile_sem_poison_stack.pop()
    assert popped is self._sem_poison
    self.nc.clear_and_free_semaphores(list(self.sems.allocated().values()))
    self.nc.all_engine_barrier()


tile.TileContext._drain_and_barrier = _patched_drain_and_barrier

# Same walrus limitation for regular engine instructions: at most one
# sync-wait per instruction.  Split extra waits onto preceding NoOps on the
# same engine (engine stalls there instead — identical semantics).
_orig_commit = tile.TileContext._commit_instruction


def _commit_split_waits(self, inst, lazy_reg_writes=True):
    si = getattr(inst, "sync_info", None)
    if (si is not None and si.on_wait and len(si.on_wait) > 1
            and inst.engine != mybir.EngineType.Unassigned):
        waits = list(si.on_wait)
        for w in waits[:-1]:
            nop = mybir.InstNoOp(
                name=self.nc.get_next_instruction_name(),
                sync_info=mybir.SyncInfo(on_wait=[w], on_update=[]),
                bass_nofuse=True,
                engine=inst.engine,
            )
            _orig_commit(self, nop, lazy_reg_writes=False)
        si.on_wait = waits[-1:]
        inst.sync_info = si
    return _orig_commit(self, inst, lazy_reg_writes)


tile.TileContext._commit_instruction = _commit_split_waits


# ---------------------------------------------------------------------------
# Config
# ---------------------------------------------------------------------------
def make_cfg(B=128, NF=40, TD=27, V=16000, DV=2048, DH=512, DW=512,
             n_cores=8, has_out_b=False, chain_mode="fp32"):
    cfg = dict(B=B, NF=NF, TD=TD, V=V, DV=DV, DH=DH, DW=DW,
               n_cores=n_cores, has_out_b=has_out_b, chain_mode=chain_mode)
    cfg["BS"] = B // n_cores          # batch rows per core
    cfg["KV"] = DV // 128             # x feature chunks
    cfg["KH"] = DH // 128             # h feature chunks
    cfg["KW"] = DW // 128             # word feature chunks
    cfg["MC"] = 3 * DH // 128         # gate chunks
    cfg["NSTEP"] = NF + TD            # total chain steps
    cfg["ROWS_E"] = NF * cfg["BS"]    # encoder (t,b) columns
    cfg["ROWS_A"] = cfg["NSTEP"] * cfg["BS"]
    cfg["ROWS_D"] = TD * cfg["BS"]    # decode (t,b) columns
    # vocab tiling for the projection (psum free dim <= 512 fp32)
    for pn in (512, 500, 400, 320, 256):
        if V % pn == 0:
            cfg["PN"] = pn
            break
    else:
        raise ValueError(f"V={V} has no tile size")
    cfg["VCH"] = V // 4               # log_softmax streaming chunk
    return cfg


def _ntiles(total, maxn):
    """Split `total` into tiles of at most maxn (last ragged)."""
    out = []
    n0 = 0
    while n0 < total:
        nn = min(maxn, total - n0)
        out.append((n0, nn))
        n0 += nn
    return out


# ---------------------------------------------------------------------------
# Kernel builder
# ---------------------------------------------------------------------------
def build_nc(cfg):
    BS, KV, KH, KW, MC = cfg["BS"], cfg["KV"], cfg["KH"], cfg["KW"], cfg["MC"]
    NF, TD, V, DH = cfg["NF"], cfg["TD"], cfg["V"], cfg["DH"]
    NSTEP, ROWS_E, ROWS_D = cfg["NSTEP"], cfg["ROWS_E"], cfg["ROWS_D"]
    PN = cfg["PN"]
    G3 = 3 * DH
    LAG = 13        # h2 chain trails h1 (> e3 block size + drain spread)
    E1N = 160       # E1 column tile = E1S chain steps
    E1S = E1N // BS
    ET = ROWS_E // E1N

    nc = bass.Bass()

    # ---- DRAM parameters (per-core views; host prepares these) ----
    xT = nc.dram_tensor("xT", [cfg["DV"], ROWS_E], BF16, kind="ExternalInput")
    wordsT = nc.dram_tensor("wordsT", [cfg["DW"], ROWS_D], BF16, kind="ExternalInput")
    w1T = nc.dram_tensor("w1T", [cfg["DV"], G3], BF16, kind="ExternalInput")
    chain_wdt = {"fp8": F8, "bf16": BF16, "fp32": F32}[cfg["chain_mode"]]
    chain_hdt = F32 if cfg["chain_mode"] == "fp32" else BF16
    wh1T = nc.dram_tensor("wh1T", [DH, G3], chain_wdt, kind="ExternalInput")
    w2T = nc.dram_tensor("w2T", [DH + cfg["DW"], G3], BF16, kind="ExternalInput")
    wh2T = nc.dram_tensor("wh2T", [DH, G3], chain_wdt, kind="ExternalInput")
    owT = nc.dram_tensor("owT", [DH, V], BF16, kind="ExternalInput")
    bi1c = nc.dram_tensor("bi1c", [128, MC], F32, kind="ExternalInput")
    bi2c = nc.dram_tensor("bi2c", [128, MC], F32, kind="ExternalInput")
    gidec = nc.dram_tensor("gidec", [128, MC, BS], BF16, kind="ExternalInput")
    bhn1 = nc.dram_tensor("bhn1", [128, KH, BS], BF16, kind="ExternalInput")
    bhn2 = nc.dram_tensor("bhn2", [128, KH, BS], BF16, kind="ExternalInput")
    ident = nc.dram_tensor("ident", [128, 128], BF16, kind="ExternalInput")
    if cfg["has_out_b"]:
        outb = nc.dram_tensor("outb", [1, V], BF16, kind="ExternalInput")
        ones = nc.dram_tensor("ones", [1, 128], BF16, kind="ExternalInput")
    out = nc.dram_tensor("out", [BS, TD, V], F32, kind="ExternalOutput")
    # view [t, b, v] of out[b, t, v] (strides V, TD*V, 1); row r = t*BS + b
    _o = out[:]
    out_tbv = bass.AP(tensor=_o.tensor, offset=_o.offset,
                      ap=[[V, TD], [TD * V, BS], [1, V]])

    def out_slice(r0, mrows, c0, cw):
        assert r0 % BS == 0 and mrows % BS == 0
        return out_tbv[r0 // BS:(r0 + mrows) // BS, :, c0:c0 + cw]

    from collections import deque
    from contextlib import ExitStack

    with tile.TileContext(nc) as tc, ExitStack() as outer_es:
        pconst = outer_es.enter_context(tc.tile_pool(name="pconst", bufs=1))
        pchain = outer_es.enter_context(tc.tile_pool(name="pchain", bufs=4))
        ph2 = outer_es.enter_context(tc.tile_pool(name="ph2", bufs=1))
        pwst = outer_es.enter_context(tc.tile_pool(name="pwst", bufs=3))
        pstage_e = outer_es.enter_context(tc.tile_pool(name="pstage_e", bufs=3))
        psmall = outer_es.enter_context(tc.tile_pool(name="psmall", bufs=2))
        psum_p = outer_es.enter_context(
            tc.tile_pool(name="psum_p", bufs=2, space="PSUM"))

        # ---- constants ----
        bi1c_sb = pconst.tile([128, MC], F32, tag="bi1c")
        nc.sync.dma_start(out=bi1c_sb[:], in_=bi1c[:])
        bi2c_sb = pconst.tile([128, MC], F32, tag="bi2c")
        nc.sync.dma_start(out=bi2c_sb[:], in_=bi2c[:])
        gidec_sb = pconst.tile([128, MC, BS], BF16, tag="gidec")
        nc.sync.dma_start(out=gidec_sb[:], in_=gidec[:])
        bhn1_sb = pconst.tile([128, KH, BS], BF16, tag="bhn1")
        nc.sync.dma_start(out=bhn1_sb[:], in_=bhn1[:])
        bhn2_sb = pconst.tile([128, KH, BS], BF16, tag="bhn2")
        nc.sync.dma_start(out=bhn2_sb[:], in_=bhn2[:])
        ident_sb = pconst.tile([128, 128], BF16, tag="ident")
        nc.sync.dma_start(out=ident_sb[:], in_=ident[:])
        if cfg["has_out_b"]:
            outb_sb = pconst.tile([1, V], BF16, tag="outb")
            nc.sync.dma_start(out=outb_sb[:], in_=outb[:])
            ones_sb = pconst.tile([1, 128], BF16, tag="ones")
            nc.sync.dma_start(out=ones_sb[:], in_=ones[:])

        h2_sb = ph2.tile([128, KH, (NSTEP + 1) * BS], chain_hdt, tag="h2")
        nc.vector.memset(h2_sb[:, :, 0:BS], 0.0)
        if cfg["chain_mode"] == "fp32":
            h2b_sb = ph2.tile([128, KH, (NSTEP + 1) * BS], BF16, tag="h2b")
            nc.vector.memset(h2b_sb[:, :, 0:BS], 0.0)
        else:
            h2b_sb = h2_sb

        # ---- projection shared state ----
        owT_r = owT[:].rearrange("(k p) n -> p k n", p=128)
        nvt = V // PN
        dcol0 = (NF + 1) * BS  # first decode h2 col
        mtiles = _ntiles(ROWS_D, 128)
        NMT = len(mtiles)
        logits_t = [None] * NMT
        sums_t = [psmall.tile([128, nvt], F32, tag=f"sums{i}", name=f"sums{i}")
                  for i in range(NMT)]
        dmaq = [nc.sync]
        qi = [0]

        def proj_unit(nt_i, wst, mt, hsrc, hc0, psum_pool):
            r0, mrows = mtiles[mt]
            n0 = nt_i * PN
            ps = psum_pool.tile([128, 512], F32, tag="pmm")
            last = KH - 1 if not cfg["has_out_b"] else None
            for k in range(KH):
                nc.tensor.matmul(
                    ps[:mrows, :PN],
                    lhsT=hsrc[:, k, hc0:hc0 + mrows],
                    rhs=wst[:, k, :],
                    start=(k == 0), stop=(k == last))
            if cfg["has_out_b"]:
                nc.tensor.matmul(
                    ps[:mrows, :PN],
                    lhsT=ones_sb[:, :mrows],
                    rhs=outb_sb[:, n0:n0 + PN],
                    start=False, stop=True)
            edump = pstage_e.tile([128, PN], BF16, tag="edump")
            nc.scalar.activation(
                out=edump[:mrows, :], in_=ps[:mrows, :PN], func=AF.Exp,
                accum_out=sums_t[mt][:mrows, nt_i:nt_i + 1])
            nc.vector.tensor_copy(
                out=logits_t[mt][:mrows, n0:n0 + PN], in_=ps[:mrows, :PN])

        def tail_chunks(mt, nch, pstage_s):
            """Emitters: lse first, then nch subtract+DMA chunks."""
            r0, mrows = mtiles[mt]
            cw = V // nch
            nshift = psmall.tile([128, 1], F32, tag=f"nshift{mt}",
                                 name=f"nshift{mt}")

            def lse():
                s1 = psmall.tile([128, 1], F32, tag=f"s1_{mt}",
                                 name=f"s1_{mt}")
                nc.vector.tensor_reduce(
                    out=s1[:mrows], in_=sums_t[mt][:mrows, :],
                    axis=mybir.AxisListType.X, op=AluOpType.add)
                nc.scalar.activation(
                    out=nshift[:mrows], in_=s1[:mrows], func=AF.Ln)
                nc.vector.tensor_scalar_mul(
                    nshift[:mrows], nshift[:mrows], -1.0)
            yield lse
            for c in range(nch):
                def sub(c=c):
                    stage = pstage_s.tile([128, cw], F32, tag="stage")
                    src = logits_t[mt][:mrows, c * cw:(c + 1) * cw]
                    if c % 2 == 0:
                        nc.scalar.activation(
                            out=stage[:mrows, :], in_=src,
                            func=AF.Identity, bias=nshift[:mrows])
                    else:
                        nc.vector.tensor_scalar_add(
                            stage[:mrows, :], src, nshift[:mrows])
                    dmaq[qi[0] % len(dmaq)].dma_start(
                        out=out_slice(r0, mrows, c * cw, cw),
                        in_=stage[:mrows, :])
                    qi[0] += 1
                yield sub

        # ---------------- gate math shared by both chains -------------
        # The gate-input biases (gi for r/z, b_hh_n for n) are seeded into
        # the PSUM accumulator by two identity matmuls, so the PE delivers
        # gi + W_hh @ h directly: the sigmoid reads PSUM as soon as the r/z
        # chunks finish (no DVE pre-op, one less cross-engine hop, and no
        # FIFO head-of-line hazard on the critical path).
        def gru_step(t, save_sb, gh, gi, shadow_sb=None):
            """gh: [128, MC, BS] psum (= seeded gi + W_hh @ h).
            gi: [128, MC, BS] (n-chunks read for the np1 add).
            save_sb holds hT; block t is h_{t-1}, writes block t+1."""
            prev = save_sb[:, :, t * BS:(t + 1) * BS]
            rzs = pchain.tile([128, 2 * KH, BS], F32, tag="rzs")
            nc.scalar.activation(out=rzs[:], in_=gh[:, 0:2 * KH, :],
                                 func=AF.Sigmoid)
            np0 = pchain.tile([128, KH, BS], F32, tag="np0")
            nc.vector.tensor_tensor(
                out=np0[:], in0=gh[:, 2 * KH:, :], in1=rzs[:, 0:KH, :],
                op=AluOpType.mult)
            np1 = pchain.tile([128, KH, BS], F32, tag="np1")
            nc.vector.tensor_tensor(
                out=np1[:], in0=gi[:, 2 * KH:, :], in1=np0[:],
                op=AluOpType.add)
            nt = pchain.tile([128, KH, BS], F32, tag="nt")
            nc.scalar.activation(out=nt[:], in_=np1[:], func=AF.Tanh)
            hm0 = pchain.tile([128, KH, BS], F32, tag="hm0")
            nc.vector.tensor_tensor(
                out=hm0[:], in0=prev[:], in1=nt[:], op=AluOpType.subtract)
            hm1 = pchain.tile([128, KH, BS], F32, tag="hm1")
            nc.vector.tensor_tensor(
                out=hm1[:], in0=rzs[:, KH:, :], in1=hm0[:], op=AluOpType.mult)
            nc.vector.tensor_tensor(
                out=save_sb[:, :, (t + 1) * BS:(t + 2) * BS],
                in0=nt[:], in1=hm1[:], op=AluOpType.add)
            if shadow_sb is not None:
                nc.vector.tensor_copy(
                    out=shadow_sb[:, :, (t + 1) * BS:(t + 2) * BS],
                    in_=save_sb[:, :, (t + 1) * BS:(t + 2) * BS])

        # ====== E1 / chains / E3 / in-chain projection ======
        with ExitStack() as chain_es:
            psum_mm = chain_es.enter_context(
                tc.tile_pool(name="psum_mm", bufs=2, space="PSUM"))
            psum_gh = chain_es.enter_context(
                tc.tile_pool(name="psum_gh", bufs=3, space="PSUM"))
            pmidA = chain_es.enter_context(tc.tile_pool(name="pmidA", bufs=1))
            h1_sb = pmidA.tile([128, KH, (NSTEP + 1) * BS], chain_hdt, tag="h1")
            nc.vector.memset(h1_sb[:, :, 0:BS], 0.0)
            if cfg["chain_mode"] == "fp32":
                h1b_sb = pmidA.tile([128, KH, (NSTEP + 1) * BS], BF16,
                                    tag="h1b")
                nc.vector.memset(h1b_sb[:, :, 0:BS], 0.0)
            else:
                h1b_sb = h1_sb
            wh1_sb = pmidA.tile([128, KH, G3], chain_wdt, tag="wh1")
            wh1T_r = wh1T[:].rearrange("(k p) n -> p k n", p=128)
            for k in range(KH):
                nc.sync.dma_start(out=wh1_sb[:, k, :], in_=wh1T_r[:, k, :])
            gi1_tiles = [pmidA.tile([128, MC, E1N], BF16, tag=f"gi1_{j}",
                                    name=f"gi1_{j}") for j in range(ET)]

            def recur_matmul(whh_sb, save_sb, t, gi, bhn_sb):
                gh = psum_gh.tile([128, MC, BS], F32, tag="gh")
                prev = save_sb[:, :, t * BS:(t + 1) * BS]
                # seed the accumulator: r/z chunks with gi, n chunks with
                # b_hh_n (the one GRU bias that sits inside the r-multiply)
                nc.tensor.matmul(
                    gh[:, 0:2 * KH, :], lhsT=ident_sb[:],
                    rhs=gi[:, 0:2 * KH, :], start=True, stop=False)
                nc.tensor.matmul(
                    gh[:, 2 * KH:, :], lhsT=ident_sb[:],
                    rhs=bhn_sb[:], start=True, stop=False)
                for m in range(MC):
                    for k in range(KH):
                        nc.tensor.matmul(
                            gh[:, m, :],
                            lhsT=whh_sb[:, k, m * 128:(m + 1) * 128],
                            rhs=prev[:, k, :],
                            start=False, stop=(k == KH - 1))
                return gh

            def h1_step(t):
                gi = (gi1_tiles[t // E1S][:, :, (t % E1S) * BS:
                                          (t % E1S + 1) * BS]
                      if t < NF else gidec_sb[:])
                gh = recur_matmul(wh1_sb, h1_sb, t, gi, bhn1_sb)
                gru_step(t, h1_sb, gh, gi,
                         h1b_sb if h1b_sb is not h1_sb else None)

            # layer-2 weights / words / gi2
            pmidB = chain_es.enter_context(tc.tile_pool(name="pmidB", bufs=1))
            w2_sb = pmidB.tile([128, KH + KW, G3], BF16, tag="w2")
            w2T_r = w2T[:].rearrange("(k p) n -> p k n", p=128)
            for k in range(KH + KW):
                nc.sync.dma_start(out=w2_sb[:, k, :], in_=w2T_r[:, k, :])
            words_sb = pmidB.tile([128, KW, ROWS_D], BF16, tag="words")
            wordsT_r = wordsT[:].rearrange("(k p) n -> p k n", p=128)
            for k in range(KW):
                nc.sync.dma_start(out=words_sb[:, k, :], in_=wordsT_r[:, k, :])
            wh2_sb = pmidB.tile([128, KH, G3], chain_wdt, tag="wh2")
            wh2T_r = wh2T[:].rearrange("(k p) n -> p k n", p=128)
            for k in range(KH):
                nc.sync.dma_start(out=wh2_sb[:, k, :], in_=wh2T_r[:, k, :])
            gi2_sb = pmidB.tile([128, MC, NSTEP * BS], BF16, tag="gi2")

            def h2_step(t):
                gi = gi2_sb[:, :, t * BS:(t + 1) * BS]
                gh = recur_matmul(wh2_sb, h2_sb, t, gi, bhn2_sb)
                gru_step(t, h2_sb, gh, gi,
                         h2b_sb if h2b_sb is not h2_sb else None)

            def e3_unit(t0, nn, m):
                """gi2[:, m, :] for chain steps [t0, t0+nn): matmuls + one
                bias-drain op (alternating ACT/DVE so neither engine gets a
                burst that stalls the gate chains)."""
                def emit():
                    n0 = t0 * BS
                    cols = nn * BS
                    enc = t0 < NF
                    ps = psum_mm.tile([128, 256], F32, tag="mm")
                    for k in range(KH):
                        nc.tensor.matmul(
                            ps[:, :cols],
                            lhsT=w2_sb[:, k, m * 128:(m + 1) * 128],
                            rhs=h1b_sb[:, k, BS + n0:BS + n0 + cols],
                            start=(k == 0),
                            stop=(enc and k == KH - 1))
                    if not enc:
                        w0 = n0 - ROWS_E
                        for k in range(KW):
                            nc.tensor.matmul(
                                ps[:, :cols],
                                lhsT=w2_sb[:, KH + k, m * 128:(m + 1) * 128],
                                rhs=words_sb[:, k, w0:w0 + cols],
                                start=False, stop=(k == KW - 1))
                    if m % 2 == 0:
                        nc.scalar.activation(
                            out=gi2_sb[:, m, n0:n0 + cols], in_=ps[:, :cols],
                            func=AF.Identity, bias=bi2c_sb[:, m:m + 1],
                            scale=1.0)
                    else:
                        nc.vector.tensor_scalar_add(
                            gi2_sb[:, m, n0:n0 + cols], ps[:, :cols],
                            bi2c_sb[:, m:m + 1])
                return emit

            def e1_unit(j, m):
                def emit():
                    n0 = j * E1N
                    ps = psum_mm.tile([128, 256], F32, tag="mm")
                    for k in range(KV):
                        nc.tensor.matmul(
                            ps[:, :E1N],
                            lhsT=w1_sb[:, k, m * 128:(m + 1) * 128],
                            rhs=x_sb[:, k, n0:n0 + E1N],
                            start=(k == 0), stop=(k == KV - 1))
                    if m % 2 == 0:
                        nc.scalar.activation(
                            out=gi1_tiles[j][:, m, :], in_=ps[:, :E1N],
                            func=AF.Identity, bias=bi1c_sb[:, m:m + 1],
                            scale=1.0)
                    else:
                        nc.vector.tensor_scalar_add(
                            gi1_tiles[j][:, m, :], ps[:, :E1N],
                            bi1c_sb[:, m:m + 1])
                return emit

            def pump(q, n):
                while n > 0 and q:
                    q.popleft()[1]()
                    n -= 1

            def flush_due(q, tt):
                while q and q[0][0] <= tt:
                    q.popleft()[1]()

            e1q = deque()
            e3q = deque()
            pq = deque()
            wst_tiles = {}

            def p_dma(nt_i):
                w = pwst.tile([128, KH, PN], BF16, tag="wst")
                nc.gpsimd.dma_start(
                    out=w[:], in_=owT_r[:, :, nt_i * PN:(nt_i + 1) * PN])
                wst_tiles[nt_i] = w

            blocks = ([(t0, nn) for (t0, nn) in _ntiles(NF, 8)] +
                      [(NF + t0, nn) for (t0, nn) in _ntiles(TD, 9)])
            block_end = {t0 + nn: (t0, nn) for (t0, nn) in blocks}
            POS = NSTEP + LAG
            P_AT = NF + 8 + LAG - 1   # position after which mt0 h2 is ready

            with ExitStack() as pw1_es:
                pw1 = pw1_es.enter_context(tc.tile_pool(name="pw1", bufs=1))
                x_sb = pw1.tile([128, KV, ROWS_E], BF16, tag="x")
                xT_r = xT[:].rearrange("(k p) n -> p k n", p=128)
                for k in range(KV):
                    nc.sync.dma_start(out=x_sb[:, k, :], in_=xT_r[:, k, :])
                w1_sb = pw1.tile([128, KV, G3], BF16, tag="w1")
                w1T_r = w1T[:].rearrange("(k p) n -> p k n", p=128)
                for k in range(KV):
                    nc.sync.dma_start(out=w1_sb[:, k, :], in_=w1T_r[:, k, :])

                # gi1 tile 0 is the serial prologue; the rest stream into
                # the chain's PE gaps (deadline = the position that reads it)
                for m in range(MC):
                    e1_unit(0, m)()
                for j in range(1, ET):
                    for m in range(MC):
                        e1q.append((j * E1S, e1_unit(j, m)))

                E1_END = 31
                for tt in range(E1_END):
                    flush_due(e1q, tt)
                    flush_due(e3q, tt)
                    if tt < NSTEP:
                        h1_step(tt)
                        if tt + 1 in block_end:
                            t0, nn = block_end[tt + 1]
                            for m in range(MC):
                                e3q.append((t0 + LAG, e3_unit(t0, nn, m)))
                    pump(e3q, 3)
                    s = tt - LAG
                    if 0 <= s < NSTEP:
                        h2_step(s)
                    pump(e1q, 2 if tt < LAG else 1)
                while e1q:
                    pump(e1q, len(e1q))

            # x / w1 freed; open the in-chain projection pool in their place
            pp_mid = outer_es.enter_context(
                tc.tile_pool(name="pp_mid", bufs=1, side="right"))
            logits_t[0] = pp_mid.tile([128, V], BF16, tag="logits0",
                                      name="logits0")
            h2p = pp_mid.tile([128, KH, 128], BF16, tag="h2p", name="h2p")

            def p_unit(nt_i):
                def emit():
                    if nt_i + 2 < nvt and (nt_i + 2) not in wst_tiles:
                        p_dma(nt_i + 2)
                    proj_unit(nt_i, wst_tiles.pop(nt_i), 0, h2p, 0, psum_p)
                return emit

            for tt in range(E1_END, POS):
                flush_due(e3q, tt)
                if tt < NSTEP:
                    h1_step(tt)
                    if tt + 1 in block_end:
                        t0, nn = block_end[tt + 1]
                        for m in range(MC):
                            e3q.append((t0 + LAG, e3_unit(t0, nn, m)))
                pump(e3q, 3)
                s = tt - LAG
                if 0 <= s < NSTEP:
                    h2_step(s)
                if tt == P_AT:
                    nc.vector.tensor_copy(
                        out=h2p[:], in_=h2b_sb[:, :, dcol0:dcol0 + 128])
                    p_dma(0)
                    p_dma(1)
                    for nt_i in range(nvt):
                        pq.append((10 ** 9, p_unit(nt_i)))
                if tt > P_AT:
                    pump(pq, 2)

        # ---- post-chain projection + log_softmax ----
        with (
            tc.tile_pool(name="ppN", bufs=1) as ppN,
            tc.tile_pool(name="pstage_s", bufs=3) as pstage_s,
            tc.tile_pool(name="psum_p2", bufs=5, space="PSUM") as psum_p2,
        ):
            while pq:
                pump(pq, len(pq))
            for i in range(1, NMT):
                logits_t[i] = ppN.tile([128, V], BF16, tag=f"logits{i}",
                                       name=f"logits{i}")
            groups = [tuple(range(1, NMT - 1)), (NMT - 1,)]
            pending = deque(tail_chunks(0, 8, pstage_s))
            for gi_, g in enumerate(groups):
                npend = len(pending)
                for nt_i in range(nvt):
                    n0 = nt_i * PN
                    wst = pwst.tile([128, KH, PN], BF16, tag="wst")
                    nc.gpsimd.dma_start(out=wst[:],
                                        in_=owT_r[:, :, n0:n0 + PN])
                    for mt in g:
                        r0, mrows = mtiles[mt]
                        proj_unit(nt_i, wst, mt, h2b_sb, dcol0 + r0, psum_p2)
                    while pending and len(pending) > (
                            npend * (nvt - 1 - nt_i)) // nvt:
                        pending.popleft()()
                for mt in g:
                    nch = 16 if mt == NMT - 1 else 8
                    pending.extend(tail_chunks(mt, nch, pstage_s))
            while pending:
                pending.popleft()()
    return nc

# ---------------------------------------------------------------------------
# Host side
# ---------------------------------------------------------------------------
def _bf16(a):
    return np.ascontiguousarray(a, dtype=np.float32).astype(ml_dtypes.bfloat16)


def _f32(a):
    return np.ascontiguousarray(a, dtype=np.float32)


def prep_inputs(cfg, vid_feats, target_variable, emb, w_ih1, w_hh1, b_ih1,
                b_hh1, w_ih2, w_hh2, b_ih2, b_hh2, out_w, out_b):
    """Build per-core input maps."""
    BS, MC, KH, DH = cfg["BS"], cfg["MC"], cfg["KH"], cfg["DH"]
    TD, NC = cfg["TD"], cfg["n_cores"]

    vid_feats = np.asarray(vid_feats, dtype=np.float32)
    target_variable = np.asarray(target_variable)
    emb = np.asarray(emb, dtype=np.float32)

    # replicated tensors
    if cfg["chain_mode"] == "fp8":
        def _chain_w(a):
            f8max = float(ml_dtypes.finfo(ml_dtypes.float8_e3m4).max)
            scaled = np.clip(np.asarray(a, dtype=np.float32) * WSCALE,
                             -f8max, f8max)
            return np.ascontiguousarray(scaled).astype(ml_dtypes.float8_e3m4)
    elif cfg["chain_mode"] == "fp32":
        _chain_w = _f32
    else:
        _chain_w = _bf16
    shared = {
        "w1T": _bf16(np.asarray(w_ih1).T),
        "wh1T": _chain_w(np.asarray(w_hh1).T),
        "w2T": _bf16(np.asarray(w_ih2).T),
        "wh2T": _chain_w(np.asarray(w_hh2).T),
        "owT": _bf16(np.asarray(out_w).T),
    }
    # combined biases: b_ih (+ b_hh for the r,z chunks; the n chunk of b_hh
    # is applied inside the gate, before the r multiply)
    def comb(bi, bh):
        c = np.asarray(bi, dtype=np.float32).copy()
        c[: 2 * DH] += np.asarray(bh, dtype=np.float32)[: 2 * DH]
        return c

    c1 = comb(b_ih1, b_hh1)
    c2 = comb(b_ih2, b_hh2)
    shared["bi1c"] = _f32(c1.reshape(MC, 128).T)
    shared["bi2c"] = _f32(c2.reshape(MC, 128).T)
    shared["gidec"] = _bf16(
        np.broadcast_to(c1.reshape(MC, 128).T[:, :, None], (128, MC, BS)))
    shared["bhn1"] = _bf16(np.broadcast_to(
        np.asarray(b_hh1, np.float32)[2 * DH:].reshape(KH, 128).T[:, :, None],
        (128, KH, BS)))
    shared["bhn2"] = _bf16(np.broadcast_to(
        np.asarray(b_hh2, np.float32)[2 * DH:].reshape(KH, 128).T[:, :, None],
        (128, KH, BS)))
    shared["ident"] = _bf16(np.eye(128))
    if cfg["has_out_b"]:
        shared["outb"] = _bf16(np.asarray(out_b).reshape(1, -1))
        shared["ones"] = _bf16(np.ones((1, 128)))

    words = emb[np.asarray(target_variable[:, :TD], dtype=np.int64)]  # [B,TD,DW]

    in_maps = []
    for c in range(NC):
        sl = slice(c * BS, (c + 1) * BS)
        vs = vid_feats[sl]                      # [BS, NF, DV]
        ws = words[sl]                          # [BS, TD, DW]
        m = dict(shared)
        m["xT"] = _bf16(vs.transpose(2, 1, 0).reshape(cfg["DV"], -1))
        m["wordsT"] = _bf16(ws.transpose(2, 1, 0).reshape(cfg["DW"], -1))
        in_maps.append(m)
    return in_maps


_CACHE = {}
LAST_RESULT = None


def kernel(**inputs):
    global LAST_RESULT
    from concourse.bass_utils import run_bass_kernel_spmd

    import os

    chain_mode = os.environ.get("KERNEL_CHAIN_MODE", "bf16")
    if chain_mode == "fp8":  # seeded-psum gates assume unscaled weights
        chain_mode = "bf16"
    out_b = np.asarray(inputs["out_b"])
    has_out_b = bool(np.any(out_b))
    key = ("full", has_out_b, chain_mode)
    if key not in _CACHE:
        cfg = make_cfg(has_out_b=has_out_b, chain_mode=chain_mode)
        _CACHE[key] = (cfg, build_nc(cfg))
    cfg, nc = _CACHE[key]

    in_maps = prep_inputs(cfg, **inputs)
    res = run_bass_kernel_spmd(nc, in_maps, list(range(cfg["n_cores"])))
    LAST_RESULT = res
    outs = [res.results[c]["out"] for c in range(cfg["n_cores"])]
    return np.concatenate(outs, axis=0)  # [B, TD, V]

